# revision 70
# baseline (speedup 1.0000x reference)
"""Causal self-attention for Trainium2, 8 NeuronCores.

Sharding: tensor-parallel over heads (4 heads/core) x data-parallel over
batch (2). Core i handles batch i//4, heads 4*(i%4)..4*(i%4)+3. Each core
computes its heads' attention output and a partial output projection
(W_proj rows for its heads); the host sums the 4 partials per batch and
adds b_proj.

Device layout choices:
  - Q^T, K^T computed feature-major [dim, t] directly (lhsT = W chunk,
    rhs = x^T chunk), so attention scores come out as S^T [k, q] with k
    on partitions -- which is exactly the layout the P@V matmul needs
    as its rhs. No on-chip transposes of the O(T^2) object.
  - V computed in natural [t, dim] layout (lhsT = x^T chunk, rhs = W_v),
    which is the lhsT layout the P@V matmul needs. A ones-column is
    appended to V so the softmax denominators fall out of the same
    matmul (row 64*... of the PSUM output).
  - exp() without max subtraction: scores are q.k/8 with q,k ~ N(0,1),
    bounded well inside fp32 exp range; softmax is shift-invariant so
    the result is mathematically identical to the reference.

Causal handling: fully-masked k-chunks are skipped; on diagonal chunks
the S^T and P@V matmuls are column-trimmed to the unmasked q-range
(PSUM zero-regions are bank-row sized, so a full-width start followed
by narrower accumulates is legal), and only the 128-wide triangular
block is multiplied by a 0/1 indicator. If the runtime mask is not the
lower-tri causal mask, a general fallback multiplies by the actual mask
(DMA'd transposed) instead; an all-ones mask drops masking entirely.

Engine budget: PE does matmuls; ACT does exp; vector does QK bias-copy,
indicator mult, stash, reciprocal; gpsimd does V bias-copy and the
output-projection PSUM->SBUF casts. Softmax denominators are inverted
once per head-pair and broadcast across the 128 partitions with a tiny
[2,128] selector matmul (engines can't move data across partitions).
"""

import numpy as np

B, T, C, H = 2, 2048, 1024, 16
D = C // H            # 64 head dim
NCORES = 8
NBG = 2               # batch shards
NHG = 4               # head-group shards
HL = H // NHG         # 4 heads per core
DL = HL * D           # 256 local feature dims
NDQ = DL // 128       # 2 partition chunks of local dims
NTB = T // 512        # 4 t-chunks of 512
NKC = T // 128        # 16 key chunks of 128
NQC = T // 512        # 4 query chunks of 512
NTT = T // 128        # 16 t-tiles of 128 (proj / V)

_CACHE = {}


def _build(mode, debug_dump=False):
    """Build + compile the per-core Bass program. mode: causal|full|general."""
    import concourse.bass as bass
    import concourse.bacc as bacc
    import concourse.tile as tile
    import concourse.mybir as mybir

    f32 = mybir.dt.float32
    bf16 = mybir.dt.bfloat16
    Exp = mybir.ActivationFunctionType.Exp
    mult = mybir.AluOpType.mult
    add = mybir.AluOpType.add

    nc = bacc.Bacc(
        "TRN2", target_bir_lowering=False, debug=False, num_devices=NCORES
    )

    xT = nc.dram_tensor("xT", [C, T], bf16, kind="ExternalInput").ap()
    Wl = nc.dram_tensor("Wl", [C, 3 * DL], bf16, kind="ExternalInput").ap()
    bqk = nc.dram_tensor("bqk", [128, 2 * NDQ], f32, kind="ExternalInput").ap()
    bv = nc.dram_tensor("bv", [1, DL], f32, kind="ExternalInput").ap()
    Wp = nc.dram_tensor("Wp", [DL, C], bf16, kind="ExternalInput").ap()
    maskT = None
    if mode == "general":
        maskT = nc.dram_tensor("maskT", [T, T], bf16, kind="ExternalInput").ap()
    yp = nc.dram_tensor("yp", [T, C], bf16, kind="ExternalOutput").ap()
    dbg = {}
    if debug_dump:
        for nm, shp, dt in [
            ("ot_d", [128, NDQ, T], bf16),
        ]:
            dbg[nm] = nc.dram_tensor(nm, shp, dt, kind="ExternalOutput").ap()

    with tile.TileContext(nc) as tc:
        with (
            tc.tile_pool(name="singles", bufs=1) as singles,
            tc.tile_pool(name="xin", bufs=2) as xin,
            tc.tile_pool(name="ptiles", bufs=8) as ptiles,
            tc.tile_pool(name="small", bufs=4) as small,
            tc.tile_pool(name="outp", bufs=4) as outp,
            tc.tile_pool(name="psum", bufs=7, space="PSUM") as psum,
        ):
            def ps512(name):
                return psum.tile(
                    [128, 512], f32, name="ps512", tag="ps512", bufs=4
                )

            # ---- resident inputs ----
            # W and the first x t-chunk live in per-kc-pair tiles so the
            # first matmuls start as soon as their own pair lands, and the
            # two DMA streams dispatch on separate queues (sync for W,
            # gpsimd for x) to halve the serialized dispatch chain.
            Wl_r = Wl.rearrange("(kc p) n -> p kc n", p=128)
            x0r = xT.rearrange("(kc p) t -> p kc t", p=128)[:, :, 0:512]
            # chunks 0 and 1 load alone (short first transfers unblock the
            # first matmuls sooner), the rest in pairs
            groups = [(0, 1), (1, 1), (2, 2), (4, 2), (6, 2)]
            W_ap = [None] * 8
            x0_ap = [None] * 8
            for gi, (k0, nk) in enumerate(groups):
                wt = singles.tile(
                    [128, nk, 3 * DL], bf16, name=f"W{gi}", tag=f"W{gi}"
                )
                nc.sync.dma_start(out=wt, in_=Wl_r[:, k0 : k0 + nk, :])
                xt = xin.tile([128, nk, 512], bf16, tag=f"x0_{gi}", bufs=1)
                nc.sync.dma_start(out=xt, in_=x0r[:, k0 : k0 + nk, :])
                for j in range(nk):
                    W_ap[k0 + j] = wt[:, j, :]
                    x0_ap[k0 + j] = xt[:, j, :]

            def W_k(kc):
                return W_ap[kc]

            bqk_sb = singles.tile([128, 2 * NDQ], f32)
            nc.sync.dma_start(out=bqk_sb, in_=bqk)
            bv_row = singles.tile([1, DL], f32)
            nc.sync.dma_start(out=bv_row, in_=bv)
            bv_sb = singles.tile([128, DL], f32)
            nc.gpsimd.partition_broadcast(bv_sb, bv_row)

            # selector for broadcasting the two per-head reciprocal rows
            # across partitions 0-63 / 64-127 via a tiny matmul
            sel2 = singles.tile([2, 128], bf16)
            nc.vector.memset(sel2, 1.0)
            # sel2[r, c] = 1 iff 64r <= c < 64r+64 (two half-plane cuts)
            nc.gpsimd.affine_select(
                out=sel2, in_=sel2, compare_op=mybir.AluOpType.is_ge,
                fill=0.0, base=0, pattern=[[1, 128]], channel_multiplier=-64,
            )
            nc.gpsimd.affine_select(
                out=sel2, in_=sel2, compare_op=mybir.AluOpType.is_ge,
                fill=0.0, base=63, pattern=[[-1, 128]], channel_multiplier=64,
            )


            ind = None
            if mode == "causal":
                ind = singles.tile([128, 4, 512], bf16)
                for j in range(4):
                    nc.vector.memset(ind[:, j, :], 1.0)
                    # keep (=1.0) iff f - p - 128*j >= 0, else 0.0
                    nc.gpsimd.affine_select(
                        out=ind[:, j, :],
                        in_=ind[:, j, :],
                        compare_op=mybir.AluOpType.is_ge,
                        fill=0.0,
                        base=-128 * j,
                        pattern=[[1, 512]],
                        channel_multiplier=-1,
                    )

            # ---- resident intermediates ----
            QT = singles.tile([128, NDQ, T], bf16)   # [dim%128, dimchunk, t]
            KT = singles.tile([128, NDQ, T], bf16)
            # V plus TWO ones-columns: head hh of a pair uses cols
            # [0:D+1+hh], so its softmax denominator lands on PSUM
            # partition 64+hh -- the pair's denominators come out
            # partition-major and need no cross-partition DMA.
            V1 = singles.tile([128, NKC, HL, D + 2], bf16)
            nc.vector.memset(V1[:, :, :, D : D + 2], 1.0)
            OT = singles.tile([128, NDQ, T], bf16)
            stage_sb = singles.tile([66, NQC, NDQ, 512], f32)
            Wp_sb = singles.tile([128, NDQ, C], bf16)

            # ---- phase 1: QKV projections (as interleavable units) ----
            def p1_units(tb, xs):
                """Units for one 512-wide t-chunk of the QKV projection.
                xs(kc) -> [128, 512] AP for contraction chunk kc."""
                units = []
                for s in range(2):  # 0=Q, 1=K
                    for dq in range(NDQ):
                        def qk_u(tb=tb, s=s, dq=dq, xs=xs):
                            ps = ps512("qk")
                            col = s * DL + dq * 128
                            for kc in range(8):
                                nc.tensor.matmul(
                                    ps,
                                    lhsT=W_k(kc)[:, col : col + 128],
                                    rhs=xs(kc),
                                    start=(kc == 0),
                                    stop=(kc == 7),
                                )
                            dst = (QT if s == 0 else KT)[
                                :, dq, tb * 512 : (tb + 1) * 512
                            ]
                            nc.vector.tensor_scalar_add(
                                dst, ps,
                                bqk_sb[:, s * NDQ + dq : s * NDQ + dq + 1],
                            )
                        units.append(qk_u)
                for t4 in range(4):
                    def v_u(tb=tb, t4=t4, xs=xs):
                        tt = tb * 4 + t4
                        ps = ps512("v")
                        for kc in range(8):
                            nc.tensor.matmul(
                                ps[:, :DL],
                                lhsT=xs(kc)[:, t4 * 128 : (t4 + 1) * 128],
                                rhs=W_k(kc)[:, 2 * DL : 3 * DL],
                                start=(kc == 0),
                                stop=(kc == 7),
                            )
                        nc.vector.tensor_tensor(
                            out=V1[:, tt, :, 0:D],
                            in0=ps[:, :DL].rearrange("p (h d) -> p h d", d=D),
                            in1=bv_sb.rearrange("p (h d) -> p h d", d=D),
                            op=add,
                        )
                    units.append(v_u)
                return units

            # ---- phase 2/3 units ----
            def proj_units(qc, tail=False):
                # tail mode (last block): alternate the PSUM->SBUF casts
                # between vector and scalar and store each 512-half as its
                # own DMA so the drain finishes sooner.
                units = []
                for t4 in range(4):
                    def u(qc=qc, t4=t4):
                        tt = qc * 4 + t4
                        y_sb = outp.tile([128, C], bf16, name="y_sb")
                        for n in range(2):
                            pp = ps512("proj")
                            for dq in range(NDQ):
                                nc.tensor.matmul(
                                    pp,
                                    lhsT=OT[:, dq, tt * 128 : (tt + 1) * 128],
                                    rhs=Wp_sb[:, dq, n * 512 : (n + 1) * 512],
                                    start=(dq == 0),
                                    stop=(dq == NDQ - 1),
                                )
                            dst = y_sb[:, n * 512 : (n + 1) * 512]
                            if tail and n == 1:
                                nc.scalar.copy(dst, pp)
                            else:
                                nc.vector.tensor_copy(dst, pp)
                        nc.sync.dma_start(
                            out=yp[tt * 128 : (tt + 1) * 128, :], in_=y_sb
                        )
                    units.append(u)
                return units

            # tail-only split projection: dq=0 accumulation pre-started
            # while the last normalize's reciprocal chain runs
            pp_state = {}

            def proj_head(tt, big=False):
                # big=True: accumulate in a ps1024 tile -- those banks free
                # as soon as the last exps drain, so the matmuls start
                # right after the final PV with no stash dependency
                if big:
                    tb = psum.tile(
                        [128, 2, 512], f32, name="ps1024", tag="ps1024",
                        bufs=2,
                    )
                for n in range(2):
                    pp = tb[:, n, :] if big else ps512("projA")
                    nc.tensor.matmul(
                        pp,
                        lhsT=OT[:, 0, tt * 128 : (tt + 1) * 128],
                        rhs=Wp_sb[:, 0, n * 512 : (n + 1) * 512],
                        start=True,
                        stop=False,
                    )
                    pp_state[(tt, n)] = pp

            def proj_finish(tt):
                y_sb = outp.tile([128, C], bf16, name="y_sb")
                for n in range(2):
                    pp = pp_state.pop((tt, n))
                    nc.tensor.matmul(
                        pp,
                        lhsT=OT[:, 1, tt * 128 : (tt + 1) * 128],
                        rhs=Wp_sb[:, 1, n * 512 : (n + 1) * 512],
                        start=False,
                        stop=True,
                    )
                    dst = y_sb[:, n * 512 : (n + 1) * 512]
                    if n == 1:
                        nc.scalar.copy(dst, pp)
                    else:
                        nc.vector.tensor_copy(dst, pp)
                nc.sync.dma_start(
                    out=yp[tt * 128 : (tt + 1) * 128, :], in_=y_sb
                )

            def attn_units(qc, hp, nkc, m_sb):
                """One head-pair's attention over all k-chunks, softmax
                denominators via the ones-column of V1. On diagonal chunks
                (causal mode) the matmuls are trimmed to q >= 128*j."""
                state = {}

                def lo_of(kc):
                    if mode == "causal" and kc >= 4 * qc:
                        return 128 * (kc - 4 * qc)
                    return 0

                def emit_mm1(j):
                    lo = lo_of(j)
                    stp = psum.tile(
                        [128, 2, 512], f32, name="ps1024", tag="ps1024", bufs=2
                    )
                    for hh in range(2):
                        off = 64 * hh
                        nc.tensor.matmul(
                            stp[:, hh, lo:],
                            lhsT=KT[off : off + 64, hp, j * 128 : (j + 1) * 128],
                            rhs=QT[
                                off : off + 64, hp,
                                qc * 512 + lo : (qc + 1) * 512,
                            ],
                            start=True,
                            stop=True,
                        )
                    state.setdefault("st", {})[j] = stp

                def prologue():
                    # ops allocation is deferred to consume(0): the PSUM
                    # buffers rotate from the previous block's ops, whose
                    # release waits on that block's stash -- allocating
                    # here would stall these mm1s behind it
                    state["emitted"] = min(2, nkc)  # lookahead 1
                    for j in range(state["emitted"]):
                        emit_mm1(j)

                def consume(kc):
                    if kc == 0:
                        state["ops"] = [ps512("o"), ps512("o")]
                    ops = state["ops"]
                    stp = state["st"].pop(kc)
                    lo = lo_of(kc)
                    p2 = ptiles.tile([128, 2, 512], bf16, tag="p")
                    nc.scalar.activation(p2[:, :, lo:], stp[:, :, lo:], Exp)
                    if mode == "causal" and kc >= 4 * qc:
                        # only the 128-wide diagonal block straddles the
                        # mask; columns beyond it are fully unmasked
                        base = ind[:, kc - 4 * qc, lo : lo + 128]
                        ind2 = bass.AP(
                            tensor=base.tensor,
                            offset=base.offset,
                            ap=[base.ap[0], [0, 2], base.ap[1]],
                        )
                        nc.vector.tensor_tensor(
                            out=p2[:, :, lo : lo + 128],
                            in0=p2[:, :, lo : lo + 128],
                            in1=ind2,
                            op=mult,
                        )
                    elif mode == "general":
                        base = m_sb[:, kc, :]
                        msk2 = bass.AP(
                            tensor=base.tensor,
                            offset=base.offset,
                            ap=[base.ap[0], [0, 2], base.ap[1]],
                        )
                        nc.vector.tensor_tensor(
                            out=p2, in0=p2, in1=msk2, op=mult
                        )
                    for hh in range(2):
                        h = hp * 2 + hh
                        nc.tensor.matmul(
                            ops[hh][: D + 1 + hh, lo:],
                            lhsT=V1[:, kc, h, : D + 1 + hh],
                            rhs=p2[:, hh, lo:],
                            start=(kc == 0),
                            stop=(kc == nkc - 1),
                        )
                    if state["emitted"] < nkc:
                        emit_mm1(state["emitted"])
                        state["emitted"] += 1

                def stash():
                    # denominator rows first (partition 64+hh of each ops
                    # bank -> stage partitions 64-65, all same-partition
                    # copies), then the unnormalized output rows. In the
                    # tail the denominators and one OT cast go to ACT so
                    # the reciprocal chain overlaps the vector casts.
                    ops = state["ops"]
                    last = qc == NQC - 1 and hp == NDQ - 1
                    # start partitions must be 32-aligned, so hh=1 copies
                    # partitions 64-65 (junk + den) and hh=0 then
                    # overwrites the junk row with its denominator
                    if last:
                        # split across engines: vector does the 2-row copy
                        # while ACT (ordered after it by the WAW overlap)
                        # patches in hh=0's denominator
                        nc.vector.tensor_copy(
                            stage_sb[64:66, qc, hp, :], ops[1][D : D + 2, :]
                        )
                        nc.scalar.copy(
                            stage_sb[64:65, qc, hp, :], ops[0][D : D + 1, :]
                        )
                    else:
                        nc.vector.tensor_copy(
                            stage_sb[64:66, qc, hp, :], ops[1][D : D + 2, :]
                        )
                        nc.vector.tensor_copy(
                            stage_sb[64:65, qc, hp, :], ops[0][D : D + 1, :]
                        )
                    for hh in range(2):
                        off = 64 * hh
                        dst = OT[off : off + 64, hp, qc * 512 : (qc + 1) * 512]
                        if last and hh == 0:
                            nc.scalar.copy(dst, ops[hh][0:D, :])
                        else:
                            nc.vector.tensor_copy(dst, ops[hh][0:D, :])

                units = [prologue]
                for kc in range(nkc):
                    units.append(lambda kc=kc: consume(kc))
                units.append(stash)
                return units

            # per-(qc, head-pair) normalization, split in two so the PE
            # half (the selector-matmul broadcast) can be deferred past
            # the vector reciprocal chain and never stalls the PE queue.
            norm_state = {}

            def norm_a(qc, hp):
                # gather the denominator pair off partition 64-65 and
                # invert. Mid-kernel: free-major on partition 0 (feeds the
                # gpsimd partition-broadcast). Tail: partition-major on
                # partitions 0-1 (feeds the selector matmul; the broadcast
                # must not sit on the gpsimd queue at the very end), and
                # on the sync HW queue to dodge the gpsimd DGE drain.
                last = qc == NQC - 1 and hp == NDQ - 1
                if last:
                    sums2 = small.tile([2, 512], f32, tag="sums2", bufs=2)
                    nc.sync.dma_start(
                        out=sums2, in_=stage_sb[64:66, qc, hp, :]
                    )
                    rcp2 = small.tile([2, 512], f32, tag="rcp2", bufs=2)
                    nc.vector.reciprocal_approx_fast(out=rcp2, in_=sums2)
                    rcpb2 = small.tile([2, 512], bf16, tag="rcpb2", bufs=2)
                    nc.vector.tensor_copy(rcpb2, rcp2)
                    norm_state[(qc, hp)] = rcpb2
                else:
                    sums2 = small.tile([1, 2, 512], f32, tag="sumf", bufs=2)
                    nc.gpsimd.dma_start(
                        out=sums2, in_=stage_sb[64:66, qc, hp, :]
                    )
                    rcp2 = small.tile([1, 2, 512], f32, tag="rcpf", bufs=2)
                    nc.vector.reciprocal_approx_fast(out=rcp2, in_=sums2)
                    norm_state[(qc, hp)] = rcp2

            def norm_b(qc, hp):
                # broadcast across partitions 0-63 / 64-127 and scale OT
                # in place. Mid-kernel: two gpsimd partition-broadcasts
                # (PE-free). Tail: [2,128] selector matmul.
                r = norm_state.pop((qc, hp))
                qs = slice(qc * 512, (qc + 1) * 512)
                if qc == NQC - 1 and hp == NDQ - 1:
                    # ps1024 bank (idle at the tail): the ps512 slots are
                    # held by the pre-started dq0 projection accumulators
                    bb = psum.tile(
                        [128, 2, 512], f32, name="ps1024", tag="ps1024",
                        bufs=2,
                    )[:, 0, :]
                    nc.tensor.matmul(bb, lhsT=sel2, rhs=r, start=True, stop=True)
                    nc.vector.tensor_tensor(
                        out=OT[:, hp, qs], in0=OT[:, hp, qs], in1=bb, op=mult
                    )
                else:
                    # broadcasts must write from partition 0 (base-64
                    # writes corrupt on HW), so the odd head's reciprocal
                    # fills a full tile and OT scales in two halves
                    rb0 = small.tile([128, 512], f32, tag="rb0", bufs=2)
                    nc.gpsimd.partition_broadcast(
                        rb0[0:64, :], r[0:1, 0, :], channels=64
                    )
                    rb1 = small.tile([128, 512], f32, tag="rb1", bufs=2)
                    nc.gpsimd.partition_broadcast(
                        rb1, r[0:1, 1, :], channels=128
                    )
                    nc.vector.tensor_tensor(
                        out=OT[0:64, hp, qs], in0=OT[0:64, hp, qs],
                        in1=rb0[0:64, :], op=mult,
                    )
                    nc.vector.tensor_tensor(
                        out=OT[64:128, hp, qs], in0=OT[64:128, hp, qs],
                        in1=rb1[64:128, :], op=mult,
                    )

            # ---- schedule: staircase interleave ----
            # attn(qc) needs phase-1 chunks tb <= qc only, so phase-1(tb+1)
            # and proj(qc-1) units are injected between attention units to
            # keep the PE FIFO fed while ACT paces the exp chain.
            for u in p1_units(0, lambda kc: x0_ap[kc]):
                u()
            nc.sync.dma_start(
                out=Wp_sb, in_=Wp.rearrange("(dq p) n -> p dq n", p=128)
            )
            for qc in range(NQC):
                nkc = 4 * qc + 4 if mode == "causal" else NKC
                m_sb = None
                if mode == "general":
                    m_sb = xin.tile([128, NKC, 512], bf16, tag="mask", bufs=1)
                    nc.sync.dma_start(
                        out=m_sb,
                        in_=maskT.rearrange("(kc p) q -> p kc q", p=128)[
                            :, :, qc * 512 : (qc + 1) * 512
                        ],
                    )
                inj_early = []
                if qc + 1 < NTB:
                    x_next = xin.tile(
                        [128, 8, 512], bf16, tag="x_sb", name="x_sb"
                    )
                    def dma_u(tb=qc + 1, x_sb=x_next):
                        xr = xT.rearrange("(kc p) t -> p kc t", p=128)[
                            :, :, tb * 512 : (tb + 1) * 512
                        ]
                        for kc in range(8):
                            nc.sync.dma_start(
                                out=x_sb[:, kc, :], in_=xr[:, kc, :]
                            )
                    inj_early.append(dma_u)
                    inj_early += p1_units(
                        qc + 1, lambda kc, x_sb=x_next: x_sb[:, kc, :]
                    )
                inj_late = proj_units(qc - 1) if qc >= 1 else []
                hp_units = []
                for hp in range(NDQ):
                    units = attn_units(qc, hp, nkc, m_sb)
                    units.append(lambda qc=qc, hp=hp: norm_a(qc, hp))
                    hp_units.append(units)
                # deferred PE-side norm: prior head-pair's broadcast runs
                # 3 units into the next block, when its reciprocal is done
                if qc > 0:
                    hp_units[0].insert(
                        3, lambda qc=qc: norm_b(qc - 1, 1)
                    )
                hp_units[1].insert(3, lambda qc=qc: norm_b(qc, 0))
                if qc == NQC - 1:
                    # pre-start the dq=0 half of three projection tiles
                    # (hp0's OT is already normalized): these matmuls fill
                    # the PE while norm_b waits on the reciprocal chain
                    hp_units[1].append(lambda: proj_head(NTT - 4, big=True))
                    hp_units[1].append(lambda: proj_head(NTT - 3))
                    hp_units[1].append(lambda: proj_head(NTT - 2))
                    hp_units[1].append(lambda qc=qc: norm_b(qc, 1))
                main = hp_units[0] + hp_units[1]
                # the appended tail units (proj_head x3 + norm_b) hold all
                # ps512 slots; no injections may land after them
                ntail = 4 if qc == NQC - 1 else 0
                half = (len(main) - ntail + 1) // 2
                mid = len(main) - ntail
                for part, inj in (
                    (main[:half], inj_early),
                    (main[half:mid], inj_late),
                ):
                    k, m, j = len(part), len(inj), 0
                    for i, u in enumerate(part):
                        u()
                        take = (i + 1) * m // k - i * m // k
                        for _ in range(take):
                            inj[j]()
                            j += 1
                for u in main[mid:]:
                    u()
            proj_finish(NTT - 4)
            proj_finish(NTT - 3)
            proj_finish(NTT - 2)
            for u in proj_units(NQC - 1, tail=True)[3:]:
                u()

            if debug_dump:
                nc.sync.dma_start(out=dbg["ot_d"], in_=OT)

    nc.compile()
    return nc


def _host_prep(x, prefix_causal_mask, W_attn, b_attn, W_proj):
    """Split full inputs into 8 per-core input maps; detect mask mode."""
    scale = 1.0 / np.sqrt(np.float32(D))
    mask = np.asarray(prefix_causal_mask)
    if mask.all():
        mode = "full"
    else:
        tri = np.tril(np.ones((T, T), dtype=bool))
        if all(np.array_equal(mask[b], tri) for b in range(B)):
            mode = "causal"
        else:
            mode = "general"

    import ml_dtypes

    bf16 = ml_dtypes.bfloat16
    x = np.asarray(x, dtype=np.float32)
    W_attn = np.asarray(W_attn, dtype=np.float32)
    b_attn = np.asarray(b_attn, dtype=np.float32)
    W_proj = np.asarray(W_proj, dtype=np.float32)

    in_maps = []
    for core in range(NCORES):
        b = core // NHG
        hg = core % NHG
        lo = hg * DL
        hi = lo + DL
        xT = np.ascontiguousarray(x[b].T)  # [C, T]
        Wq = W_attn[:, lo:hi] * scale
        Wk = W_attn[:, C + lo : C + hi]
        Wv = W_attn[:, 2 * C + lo : 2 * C + hi]
        Wl = np.ascontiguousarray(np.concatenate([Wq, Wk, Wv], axis=1))
        bq = b_attn[lo:hi] * scale
        bk = b_attn[C + lo : C + hi]
        # bias per partition for Q,K chunks: cols = [q0, q1, k0, k1]
        bqk = np.stack(
            [bq[0:128], bq[128:256], bk[0:128], bk[128:256]], axis=1
        ).astype(np.float32)
        bv = np.ascontiguousarray(
            b_attn[2 * C + lo : 2 * C + hi][None, :]
        ).astype(np.float32)
        Wp = np.ascontiguousarray(W_proj[lo:hi, :])
        im = {
            "xT": xT.astype(bf16),
            "Wl": Wl.astype(bf16),
            "bqk": np.ascontiguousarray(bqk),
            "bv": bv,
            "Wp": Wp.astype(bf16),
        }
        if mode == "general":
            im["maskT"] = np.ascontiguousarray(mask[b].T).astype(bf16)
        in_maps.append(im)
    return mode, in_maps


def _get_program(mode):
    if mode not in _CACHE:
        _CACHE[mode] = _build(mode)
    return _CACHE[mode]


def _run(inputs, trace=False):
    """Returns (full_output [B,T,C], BassKernelResults)."""
    from concourse import bass_utils

    mode, in_maps = _host_prep(
        inputs["x"],
        inputs["prefix_causal_mask"],
        inputs["W_attn"],
        inputs["b_attn"],
        inputs["W_proj"],
    )
    nc = _get_program(mode)
    res = bass_utils.run_bass_kernel_spmd(
        nc, in_maps, core_ids=list(range(NCORES)), trace=trace
    )
    b_proj = np.asarray(inputs["b_proj"], dtype=np.float32)
    y = np.zeros((B, T, C), dtype=np.float32)
    for core in range(NCORES):
        y[core // NHG] += np.asarray(res.results[core]["yp"], dtype=np.float32)
    y += b_proj[None, None, :]
    return y, res


def kernel(**inputs):
    y, _ = _run(inputs, trace=False)
    return y
